# revision 26
# baseline (speedup 1.0000x reference)
"""Trainium2 Bass kernel for nn_BinaryFullTensorCell (gnn_message_passing).

Computes, for each node n:
    out[n,k] = sum_{i,j} h1[n,i]*h2[n,j]*A[i,j,k] + h1@U1_w.T + h2@U2_w.T + U2_b

Sharding: data-parallel over the node axis across 8 NeuronCores (2048
nodes/core); A / U1_w / U2_w / U2_b replicated.

Algorithm (v4, fp8e4 DoubleRow, single-e4m3-rounding error budget):
  - Host prep (layout/precision transforms of inputs only): A is
    mean-centered (A' = A - 0.5) and split into hi+lo fp8e4 terms in
    DoubleRow pair layout [j, pair, 2, k]; h1^T in bf16 broadcast-chunk
    layout; h2^T/h1^T/U1^T/U2^T in bf16.
  - outer_T[j, (c, n)] = h1T[c,n]*h2T[j,n] is built directly in
    transposed layout: h1T chunks are replicated across all 128
    partitions either by a stride-0 DMA broadcast or by GPSIMD
    PartitionBroadcast (split tunable via K_GPB), then multiplied by
    the partition-resident bf16 h2T on DVE.  A slice of c-rows is
    multiplied to bf16 at DVE 2x rate and quantized to fp8 on ACT; the
    rest is multiplied straight to fp8 on DVE (1x).  This is the only
    e4m3 rounding in the bilinear chain.
  - PE accumulates out_T[k, n] with fp8e4 DoubleRow matmuls against
    Ahi and Alo (A quantization error second order).
  - Mean restoration: bil += 0.5*s1[n]*s2[n] via g[n,j]=h2[n,j]*s1[n]
    (DVE), PE transpose, f32r matmul against a constant 0.5 matrix.
  - Linear terms as bf16 matmuls into the same PSUM group; bias added
    on ACT eviction.  Output stored transposed [3H, NS]; host
    un-transposes.
"""

import os

import numpy as np

N_FULL = 16384
N_CORES = 8
NS = N_FULL // N_CORES  # 2048 nodes per core
H = 128
KO = 3 * H  # 384
P = 128
GROUPS = 4  # groups of 512 nodes per core
GN = NS // GROUPS  # 512
NH = 1  # n-halves per group for bc/mult tiling
HN = GN // NH  # 512
NT = NS // P  # 16 node-tiles per core
CCH = 16  # contraction rows (c values) per broadcast/mult chunk
NCB = H // CCH  # 8 chunks
CPAIR = CCH // 2  # DoubleRow pairs per chunk

# Of each 16-c-row chunk, QA rows go DVE-mult(bf16,2x) -> ACT-quantize(fp8);
# the rest go DVE-mult straight to fp8 (1x).
# Per 16-c chunk: rows 0..7 bf16 path; rows 8..13 fp8 via ACT quantize;
# rows 14..15 fp8 via DVE direct multiply.
QA = int(os.environ.get("K_QA", "6"))
# Of every 16 bc tiles, K_GPB are broadcast by GPSIMD instead of DMA.
K_GPB = int(os.environ.get("K_GPB", "0"))
# PE warm-up matmuls at kernel start (DVFS ramp), overlapping initial DMAs.
WARMUP = int(os.environ.get("K_WARMUP", "0"))

_CACHE = {}


def _build_nc():
    import concourse.bacc as bacc
    import concourse.mybir as mybir
    import concourse.tile as tile
    from concourse.masks import make_identity

    f32 = mybir.dt.float32
    f32r = mybir.dt.float32r
    bf16 = mybir.dt.bfloat16
    f8 = mybir.dt.float8e4
    DR = mybir.MatmulPerfMode.DoubleRow
    AF = mybir.ActivationFunctionType
    nc = bacc.Bacc("TRN2", target_bir_lowering=False, debug=False)

    a16 = nc.dram_tensor("a16", [P, NCB, 8, KO], bf16, kind="ExternalInput")
    a8p = nc.dram_tensor("a8p", [P, NCB, 4, 2, KO], f8, kind="ExternalInput")
    # h1bc[cb, g, h, c, n] = h1T[cb*16+c, g*1024 + h*512 + n] (bf16)
    h1bc = nc.dram_tensor(
        "h1bc", [NCB, GROUPS, NH, CCH, HN], bf16, kind="ExternalInput"
    )
    h2tb = nc.dram_tensor("h2tb", [H, NS], bf16, kind="ExternalInput")
    h1tb = nc.dram_tensor("h1tb", [H, NS], bf16, kind="ExternalInput")
    u1t = nc.dram_tensor("u1t", [H, 3, P], bf16, kind="ExternalInput")
    u2t = nc.dram_tensor("u2t", [H, 3, P], bf16, kind="ExternalInput")
    biasw = nc.dram_tensor("biasw", [P, 3], f32, kind="ExternalInput")
    nh = nc.dram_tensor("neighbour_h", [NS, 2, H], f32, kind="ExternalInput")
    outT = nc.dram_tensor("outT", [KO, NS], f32, kind="ExternalOutput")

    with tile.TileContext(nc) as tc:
        with tc.tile_pool(name="consts", bufs=1) as consts:
            identity_r = consts.tile([P, P], f32r)
            half_sb = consts.tile([P, P], f32r)
            with tc.tile_pool(name="tmpconst", bufs=1) as tmpc:
                identity = tmpc.tile([P, P], f32)
                make_identity(nc, identity)
                nc.vector.tensor_copy(identity_r[:], identity[:])
                half_f32 = tmpc.tile([P, P], f32)
                nc.vector.memset(half_f32[:], 0.5)
                nc.vector.tensor_copy(half_sb[:], half_f32[:])

            h2tb_sb = consts.tile([P, NS], bf16)
            for q in range(2):
                nc.sync.dma_start(
                    out=h2tb_sb[:, q * 1024 : (q + 1) * 1024],
                    in_=h2tb.ap()[:, q * 1024 : (q + 1) * 1024],
                )
            h1tb_sb = consts.tile([P, NS], bf16)
            for q in range(2):
                nc.sync.dma_start(
                    out=h1tb_sb[:, q * 1024 : (q + 1) * 1024],
                    in_=h1tb.ap()[:, q * 1024 : (q + 1) * 1024],
                )
            u1t_sb = consts.tile([P, 3, P], bf16)
            nc.sync.dma_start(out=u1t_sb[:], in_=u1t.ap())
            u2t_sb = consts.tile([P, 3, P], bf16)
            nc.sync.dma_start(out=u2t_sb[:], in_=u2t.ap())
            bias_sb = consts.tile([P, 3], f32)
            nc.sync.dma_start(out=bias_sb[:], in_=biasw.ap())
            h_sb = consts.tile([P, NT, 2, H], f32)
            _nh_r = nh.ap().rearrange("(t p) a b -> p t a b", p=P)
            for q in range(4):
                nc.sync.dma_start(
                    out=h_sb[:, q * 4 : (q + 1) * 4], in_=_nh_r[:, q * 4 : (q + 1) * 4]
                )

            s1_sb = consts.tile([P, NT, 1], f32)
            g_sb = consts.tile([P, NT, H], f32r)
            gt_sb = consts.tile([P, NS], f32)

            with (
                tc.tile_pool(name="alo_pool", bufs=2) as alo_pool,
                tc.tile_pool(name="bc_pool", bufs=2) as bc_pool,
                tc.tile_pool(name="stage_pool", bufs=2) as stage_pool,
                tc.tile_pool(name="obf_pool", bufs=2) as obf_pool,
                tc.tile_pool(name="o8_pool", bufs=3) as o8_pool,
                tc.tile_pool(name="acc_ps", bufs=6, space="PSUM") as acc_ps,
                tc.tile_pool(name="tr_ps", bufs=2, space="PSUM") as tr_ps,
                tc.tile_pool(name="osb_pool", bufs=2) as osb_pool,
            ):
                def _emit_setup():
                    nc.vector.tensor_reduce(
                        s1_sb[:],
                        h_sb[:, :, 0, :],
                        mybir.AxisListType.X,
                        mybir.AluOpType.add,
                    )
                    nc.vector.tensor_mul(
                        g_sb[:],
                        h_sb[:, :, 1, :],
                        s1_sb[:].broadcast_to([P, NT, H]),
                    )
                    for tq in range(NT // 4):
                        ps = tr_ps.tile([P, 512], f32r, tag="tr", name=f"gt{tq}")
                        for tt in range(4):
                            t = tq * 4 + tt
                            nc.tensor.transpose(
                                ps[:, tt * P : (tt + 1) * P],
                                g_sb[:, t, :],
                                identity_r[:],
                            )
                        nc.vector.tensor_copy(
                            gt_sb[:, tq * 512 : (tq + 1) * 512].bitcast(f32r), ps[:]
                        )

                # PE warm-up: back-to-back DoubleRow matmuls on constant
                # data to push DVFS to the high p-state while the initial
                # DMAs land.  Results are discarded (overwritten by start=True
                # of the real accumulation later via pool reuse).
                if WARMUP:
                    wsrc = consts.tile([P, 2, 256], f8)
                    nc.vector.memset(wsrc[:], 0.25)
                    wps = tr_ps.tile([P, 512], f32, tag="tr", name="warm")
                    for w in range(WARMUP):
                        nc.tensor.matmul(
                            wps[:, :256],
                            wsrc[:, :, :P],
                            wsrc[:],
                            start=True,
                            stop=True,
                            perf_mode=DR,
                        )

                bc_idx = 0
                for g in range(GROUPS):
                    acc = [
                        acc_ps.tile([P, GN], f32, tag="acc", name=f"acc{g}_{m}")
                        for m in range(3)
                    ]
                    for cb in range(NCB):
                        a16t = alo_pool.tile([P, 8, KO], bf16, tag="a16")
                        for q in range(4):
                            nc.sync.dma_start(
                                out=a16t[:, q * 2 : (q + 1) * 2],
                                in_=a16.ap()[:, cb, q * 2 : (q + 1) * 2],
                            )
                        a8t = alo_pool.tile([P, 4, 2, KO], f8, tag="a8")
                        for q in range(2):
                            nc.sync.dma_start(
                                out=a8t[:, q * 2 : (q + 1) * 2],
                                in_=a8p.ap()[:, cb, q * 2 : (q + 1) * 2],
                            )
                        for hh in range(NH):
                            n0 = g * GN + hh * HN
                            bc = bc_pool.tile([P, CCH, HN], bf16, tag="bc")
                            if bc_idx % 16 < K_GPB:
                                HL = CCH * HN // 2
                                for sh in range(2):
                                    stage = stage_pool.tile([1, HL], bf16, tag="st")
                                    nc.sync.dma_start(
                                        out=stage[:],
                                        in_=h1bc.ap()[cb, g, hh]
                                        .rearrange("c n -> (c n)")
                                        .unsqueeze(0)[:, sh * HL : (sh + 1) * HL],
                                    )
                                    nc.gpsimd.partition_broadcast(
                                        bc[:].rearrange("p c n -> p (c n)")[
                                            :, sh * HL : (sh + 1) * HL
                                        ],
                                        stage[0:1, :],
                                    )
                            else:
                                for q in range(4):
                                    nc.sync.dma_start(
                                        out=bc[:, q * 4 : (q + 1) * 4, :],
                                        in_=h1bc.ap()[cb, g, hh][
                                            q * 4 : (q + 1) * 4
                                        ]
                                        .unsqueeze(0)
                                        .broadcast_to([P, 4, HN]),
                                    )
                            bc_idx += 1
                            # bf16 path: rows 0..7 feed PE directly as bf16
                            ob16 = obf_pool.tile([P, 8, HN], bf16, tag="ob16")
                            nc.vector.tensor_mul(
                                ob16[:],
                                bc[:, :8, :],
                                h2tb_sb[:, None, n0 : n0 + HN].broadcast_to(
                                    [P, 8, HN]
                                ),
                            )
                            # fp8 path: rows 8..13 via bf16+ACT, 14..15 direct
                            o8 = o8_pool.tile([P, 8, HN], f8, tag="o8")
                            obf = obf_pool.tile([P, QA, HN], bf16, tag="obf")
                            nc.vector.tensor_mul(
                                obf[:],
                                bc[:, 8 : 8 + QA, :],
                                h2tb_sb[:, None, n0 : n0 + HN].broadcast_to(
                                    [P, QA, HN]
                                ),
                            )
                            nc.scalar.copy(o8[:, :QA, :], obf[:])
                            nc.vector.tensor_mul(
                                o8[:, QA:, :],
                                bc[:, 8 + QA :, :],
                                h2tb_sb[:, None, n0 : n0 + HN].broadcast_to(
                                    [P, 8 - QA, HN]
                                ),
                            )
                            for cl in range(8):
                                c = cb * CCH + cl
                                for m in range(3):
                                    nc.tensor.matmul(
                                        acc[m][:, hh * HN : (hh + 1) * HN],
                                        a16t[:, cl, m * P : (m + 1) * P],
                                        ob16[:, cl, :],
                                        start=(c == 0),
                                        stop=False,
                                    )
                            for pp in range(4):
                                for m in range(3):
                                    nc.tensor.matmul(
                                        acc[m][:, hh * HN : (hh + 1) * HN],
                                        a8t[:, pp, :, m * P : (m + 1) * P],
                                        o8[:, 2 * pp : 2 * pp + 2, :],
                                        start=False,
                                        stop=False,
                                        perf_mode=DR,
                                    )
                        if g == 0 and cb == 0:
                            _emit_setup()
                    # Linear terms, mean correction, bias, store (transposed).
                    n0 = g * GN
                    for m in range(3):
                        for hh in range(NH):
                            nn = n0 + hh * HN
                            sl = acc[m][:, hh * HN : (hh + 1) * HN]
                            nc.tensor.matmul(
                                sl,
                                u1t_sb[:, m, :],
                                h1tb_sb[:, nn : nn + HN],
                                start=False,
                                stop=False,
                            )
                            nc.tensor.matmul(
                                sl,
                                u2t_sb[:, m, :],
                                h2tb_sb[:, nn : nn + HN],
                                start=False,
                                stop=False,
                            )
                            nc.tensor.matmul(
                                sl,
                                half_sb[:],
                                gt_sb[:, nn : nn + HN].bitcast(f32r),
                                start=False,
                                stop=True,
                            )
                        osb = osb_pool.tile([P, GN], f32, tag="osb")
                        nc.scalar.activation(
                            osb[:], acc[m][:], AF.Identity, bias=bias_sb[:, m : m + 1]
                        )
                        nc.sync.dma_start(
                            out=outT.ap()[m * P : (m + 1) * P, n0 : n0 + GN],
                            in_=osb[:],
                        )

    nc.compile()
    return nc


def _get_nc():
    if "nc" not in _CACHE:
        _CACHE["nc"] = _build_nc()
    return _CACHE["nc"]


def _prep_full(inputs):
    import ml_dtypes

    f8 = ml_dtypes.float8_e4m3
    bf = ml_dtypes.bfloat16
    nhf = np.ascontiguousarray(np.asarray(inputs["neighbour_h"], dtype=np.float32))
    A = np.asarray(inputs["A"], dtype=np.float32)
    U1 = np.asarray(inputs["U1_w"], dtype=np.float32)
    U2 = np.asarray(inputs["U2_w"], dtype=np.float32)
    U2b = np.asarray(inputs["U2_b"], dtype=np.float32)

    Ac = (A - 0.5).astype(np.float32)
    # c-local rows 0..7 of each 16-chunk: bf16; rows 8..15: fp8 pairs
    Ac4 = Ac.reshape(NCB, CCH, H, KO)
    a16 = np.ascontiguousarray(
        Ac4[:, :8].transpose(2, 0, 1, 3)
    ).astype(bf)  # [j, cb, 8, KO]
    a8p = np.ascontiguousarray(
        Ac4[:, 8:].reshape(NCB, 4, 2, H, KO).transpose(3, 0, 1, 2, 4)
    ).astype(f8)  # [j, cb, 4, 2, KO]

    h1 = nhf[:, 0, :]
    h2 = nhf[:, 1, :]
    h1T = np.ascontiguousarray(h1.T).astype(bf)
    h2T = np.ascontiguousarray(h2.T).astype(bf)
    # h1bc[core][cb, g, h, c, n]
    h1bc = np.ascontiguousarray(
        h1T.reshape(NCB, CCH, N_CORES, GROUPS, NH, HN).transpose(2, 0, 3, 4, 1, 5)
    )

    u1t = np.ascontiguousarray(U1.reshape(3, P, H).transpose(2, 0, 1)).astype(bf)
    u2t = np.ascontiguousarray(U2.reshape(3, P, H).transpose(2, 0, 1)).astype(bf)
    biasw = np.ascontiguousarray(U2b.reshape(3, P).T)

    return nhf, a16, a8p, h1bc, h1T, h2T, u1t, u2t, biasw


def make_in_maps(inputs):
    nhf, a16, a8p, h1bc, h1T, h2T, u1t, u2t, biasw = _prep_full(inputs)
    return [
        {
            "a16": a16,
            "a8p": a8p,
            "h1bc": h1bc[i],
            "h2tb": np.ascontiguousarray(h2T[:, i * NS : (i + 1) * NS]),
            "h1tb": np.ascontiguousarray(h1T[:, i * NS : (i + 1) * NS]),
            "u1t": u1t,
            "u2t": u2t,
            "biasw": biasw,
            "neighbour_h": nhf[i * NS : (i + 1) * NS],
        }
        for i in range(N_CORES)
    ]


def kernel(**inputs: np.ndarray) -> np.ndarray:
    in_maps = make_in_maps(inputs)
    nc = _get_nc()
    from concourse import bass2jax

    results = bass2jax.run_bass_via_pjrt(nc, in_maps, n_cores=N_CORES)
    return np.concatenate(
        [np.asarray(results[i]["outT"], dtype=np.float32).T for i in range(N_CORES)],
        axis=0,
    )


if __name__ == "__main__":
    rng = np.random.default_rng(0)
    ins = {
        "neighbour_h": rng.standard_normal((N_FULL, 2, H), dtype=np.float32),
        "A": rng.random((H, H, KO), dtype=np.float32),
        "U1_w": rng.standard_normal((KO, H), dtype=np.float32),
        "U2_w": rng.standard_normal((KO, H), dtype=np.float32),
        "U2_b": rng.standard_normal((KO,), dtype=np.float32),
    }
    out = kernel(**ins)
    h1 = ins["neighbour_h"][:, 0, :].astype(np.float64)
    h2 = ins["neighbour_h"][:, 1, :].astype(np.float64)
    A = ins["A"].astype(np.float64)
    outer = np.einsum("ni,nj->nij", h1, h2).reshape(N_FULL, H * H)
    exp = (
        outer @ A.reshape(H * H, KO)
        + h1 @ ins["U1_w"].T.astype(np.float64)
        + h2 @ ins["U2_w"].T.astype(np.float64)
        + ins["U2_b"].astype(np.float64)
    )
    err = np.linalg.norm(out - exp) / np.linalg.norm(exp)
    print("kernel output", out.shape, out.dtype, "rel fro err:", err)


# revision 27
# speedup vs baseline: 1.3232x; 1.3232x over previous
"""Trainium2 Bass kernel for nn_BinaryFullTensorCell (gnn_message_passing).

Computes, for each node n:
    out[n,k] = sum_{i,j} h1[n,i]*h2[n,j]*A[i,j,k] + h1@U1_w.T + h2@U2_w.T + U2_b

Sharding: data-parallel over the node axis across 8 NeuronCores (2048
nodes/core); A / U1_w / U2_w / U2_b replicated.

Algorithm (v4, fp8e4 DoubleRow, single-e4m3-rounding error budget):
  - Host prep (layout/precision transforms of inputs only): A is
    mean-centered (A' = A - 0.5) and split into hi+lo fp8e4 terms in
    DoubleRow pair layout [j, pair, 2, k]; h1^T in bf16 broadcast-chunk
    layout; h2^T/h1^T/U1^T/U2^T in bf16.
  - outer_T[j, (c, n)] = h1T[c,n]*h2T[j,n] is built directly in
    transposed layout: h1T chunks are replicated across all 128
    partitions either by a stride-0 DMA broadcast or by GPSIMD
    PartitionBroadcast (split tunable via K_GPB), then multiplied by
    the partition-resident bf16 h2T on DVE.  A slice of c-rows is
    multiplied to bf16 at DVE 2x rate and quantized to fp8 on ACT; the
    rest is multiplied straight to fp8 on DVE (1x).  This is the only
    e4m3 rounding in the bilinear chain.
  - PE accumulates out_T[k, n] with fp8e4 DoubleRow matmuls against
    Ahi and Alo (A quantization error second order).
  - Mean restoration: bil += 0.5*s1[n]*s2[n] via g[n,j]=h2[n,j]*s1[n]
    (DVE), PE transpose, f32r matmul against a constant 0.5 matrix.
  - Linear terms as bf16 matmuls into the same PSUM group; bias added
    on ACT eviction.  Output stored transposed [3H, NS]; host
    un-transposes.
"""

import os

import numpy as np

N_FULL = 16384
N_CORES = 8
NS = N_FULL // N_CORES  # 2048 nodes per core
H = 128
KO = 3 * H  # 384
P = 128
GROUPS = 2  # groups of 1024 nodes per core
GN = NS // GROUPS  # 1024
NH = 2  # n-halves per group for bc/mult tiling
HN = GN // NH  # 512
NT = NS // P  # 16 node-tiles per core
CCH = 16  # contraction rows (c values) per broadcast/mult chunk
NCB = H // CCH  # 8 chunks
CPAIR = CCH // 2  # DoubleRow pairs per chunk

# Of each 16-c-row chunk, QA rows go DVE-mult(bf16,2x) -> ACT-quantize(fp8);
# the rest go DVE-mult straight to fp8 (1x).
# Per 16-c chunk: rows 0..7 bf16 path; rows 8..13 fp8 via ACT quantize;
# rows 14..15 fp8 via DVE direct multiply.
QA = int(os.environ.get("K_QA", "6"))
# Of every 16 bc tiles, K_GPB are broadcast by GPSIMD instead of DMA.
K_GPB = int(os.environ.get("K_GPB", "0"))
# PE warm-up matmuls at kernel start (DVFS ramp), overlapping initial DMAs.
WARMUP = int(os.environ.get("K_WARMUP", "0"))

_CACHE = {}


def _build_nc():
    import concourse.bacc as bacc
    import concourse.mybir as mybir
    import concourse.tile as tile
    from concourse.masks import make_identity

    f32 = mybir.dt.float32
    f32r = mybir.dt.float32r
    bf16 = mybir.dt.bfloat16
    f8 = mybir.dt.float8e4
    DR = mybir.MatmulPerfMode.DoubleRow
    AF = mybir.ActivationFunctionType
    nc = bacc.Bacc("TRN2", target_bir_lowering=False, debug=False)

    a16 = nc.dram_tensor("a16", [P, NCB, 8, KO], bf16, kind="ExternalInput")
    a8p = nc.dram_tensor("a8p", [P, NCB, 4, 2, KO], f8, kind="ExternalInput")
    # h1bc[cb, g, h, c, n] = h1T[cb*16+c, g*1024 + h*512 + n] (bf16)
    h1bc = nc.dram_tensor(
        "h1bc", [NCB, GROUPS, NH, CCH, HN], bf16, kind="ExternalInput"
    )
    h2tb = nc.dram_tensor("h2tb", [H, NS], bf16, kind="ExternalInput")
    h1tb = nc.dram_tensor("h1tb", [H, NS], bf16, kind="ExternalInput")
    u1t = nc.dram_tensor("u1t", [H, 3, P], bf16, kind="ExternalInput")
    u2t = nc.dram_tensor("u2t", [H, 3, P], bf16, kind="ExternalInput")
    biasw = nc.dram_tensor("biasw", [P, 3], f32, kind="ExternalInput")
    nh = nc.dram_tensor("neighbour_h", [NS, 2, H], f32, kind="ExternalInput")
    outT = nc.dram_tensor("outT", [KO, NS], f32, kind="ExternalOutput")

    with tile.TileContext(nc) as tc:
        with tc.tile_pool(name="consts", bufs=1) as consts:
            identity_r = consts.tile([P, P], f32r)
            half_sb = consts.tile([P, P], f32r)
            with tc.tile_pool(name="tmpconst", bufs=1) as tmpc:
                identity = tmpc.tile([P, P], f32)
                make_identity(nc, identity)
                nc.vector.tensor_copy(identity_r[:], identity[:])
                half_f32 = tmpc.tile([P, P], f32)
                nc.vector.memset(half_f32[:], 0.5)
                nc.vector.tensor_copy(half_sb[:], half_f32[:])

            h2tb_sb = consts.tile([P, NS], bf16)
            for q in range(2):
                nc.sync.dma_start(
                    out=h2tb_sb[:, q * 1024 : (q + 1) * 1024],
                    in_=h2tb.ap()[:, q * 1024 : (q + 1) * 1024],
                )
            h1tb_sb = consts.tile([P, NS], bf16)
            for q in range(2):
                nc.sync.dma_start(
                    out=h1tb_sb[:, q * 1024 : (q + 1) * 1024],
                    in_=h1tb.ap()[:, q * 1024 : (q + 1) * 1024],
                )
            u1t_sb = consts.tile([P, 3, P], bf16)
            nc.sync.dma_start(out=u1t_sb[:], in_=u1t.ap())
            u2t_sb = consts.tile([P, 3, P], bf16)
            nc.sync.dma_start(out=u2t_sb[:], in_=u2t.ap())
            bias_sb = consts.tile([P, 3], f32)
            nc.sync.dma_start(out=bias_sb[:], in_=biasw.ap())
            h_sb = consts.tile([P, NT, 2, H], f32)
            _nh_r = nh.ap().rearrange("(t p) a b -> p t a b", p=P)
            for q in range(4):
                nc.sync.dma_start(
                    out=h_sb[:, q * 4 : (q + 1) * 4], in_=_nh_r[:, q * 4 : (q + 1) * 4]
                )

            s1_sb = consts.tile([P, NT, 1], f32)
            g_sb = consts.tile([P, NT, H], f32r)
            gt_sb = consts.tile([P, NS], f32)

            with (
                tc.tile_pool(name="alo_pool", bufs=2) as alo_pool,
                tc.tile_pool(name="bc_pool", bufs=2) as bc_pool,
                tc.tile_pool(name="stage_pool", bufs=2) as stage_pool,
                tc.tile_pool(name="obf_pool", bufs=2) as obf_pool,
                tc.tile_pool(name="o8_pool", bufs=3) as o8_pool,
                tc.tile_pool(name="acc_ps", bufs=3, space="PSUM") as acc_ps,
                tc.tile_pool(name="tr_ps", bufs=2, space="PSUM") as tr_ps,
                tc.tile_pool(name="osb_pool", bufs=2) as osb_pool,
            ):
                def _emit_setup():
                    nc.vector.tensor_reduce(
                        s1_sb[:],
                        h_sb[:, :, 0, :],
                        mybir.AxisListType.X,
                        mybir.AluOpType.add,
                    )
                    nc.vector.tensor_mul(
                        g_sb[:],
                        h_sb[:, :, 1, :],
                        s1_sb[:].broadcast_to([P, NT, H]),
                    )
                    for tq in range(NT // 4):
                        ps = tr_ps.tile([P, 512], f32r, tag="tr", name=f"gt{tq}")
                        for tt in range(4):
                            t = tq * 4 + tt
                            nc.tensor.transpose(
                                ps[:, tt * P : (tt + 1) * P],
                                g_sb[:, t, :],
                                identity_r[:],
                            )
                        nc.vector.tensor_copy(
                            gt_sb[:, tq * 512 : (tq + 1) * 512].bitcast(f32r), ps[:]
                        )

                # PE warm-up: back-to-back DoubleRow matmuls on constant
                # data to push DVFS to the high p-state while the initial
                # DMAs land.  Results are discarded (overwritten by start=True
                # of the real accumulation later via pool reuse).
                if WARMUP:
                    wsrc = consts.tile([P, 2, 256], f8)
                    nc.vector.memset(wsrc[:], 0.25)
                    wps = tr_ps.tile([P, 512], f32, tag="tr", name="warm")
                    for w in range(WARMUP):
                        nc.tensor.matmul(
                            wps[:, :256],
                            wsrc[:, :, :P],
                            wsrc[:],
                            start=True,
                            stop=True,
                            perf_mode=DR,
                        )

                bc_idx = 0
                for g in range(GROUPS):
                    acc = [
                        acc_ps.tile([P, GN], f32, tag="acc", name=f"acc{g}_{m}")
                        for m in range(3)
                    ]
                    for cb in range(NCB):
                        a16t = alo_pool.tile([P, 8, KO], bf16, tag="a16")
                        for q in range(4):
                            nc.sync.dma_start(
                                out=a16t[:, q * 2 : (q + 1) * 2],
                                in_=a16.ap()[:, cb, q * 2 : (q + 1) * 2],
                            )
                        a8t = alo_pool.tile([P, 4, 2, KO], f8, tag="a8")
                        for q in range(2):
                            nc.sync.dma_start(
                                out=a8t[:, q * 2 : (q + 1) * 2],
                                in_=a8p.ap()[:, cb, q * 2 : (q + 1) * 2],
                            )
                        for hh in range(NH):
                            n0 = g * GN + hh * HN
                            bc = bc_pool.tile([P, CCH, HN], bf16, tag="bc")
                            if bc_idx % 16 < K_GPB:
                                HL = CCH * HN // 2
                                for sh in range(2):
                                    stage = stage_pool.tile([1, HL], bf16, tag="st")
                                    nc.sync.dma_start(
                                        out=stage[:],
                                        in_=h1bc.ap()[cb, g, hh]
                                        .rearrange("c n -> (c n)")
                                        .unsqueeze(0)[:, sh * HL : (sh + 1) * HL],
                                    )
                                    nc.gpsimd.partition_broadcast(
                                        bc[:].rearrange("p c n -> p (c n)")[
                                            :, sh * HL : (sh + 1) * HL
                                        ],
                                        stage[0:1, :],
                                    )
                            else:
                                for q in range(4):
                                    nc.sync.dma_start(
                                        out=bc[:, q * 4 : (q + 1) * 4, :],
                                        in_=h1bc.ap()[cb, g, hh][
                                            q * 4 : (q + 1) * 4
                                        ]
                                        .unsqueeze(0)
                                        .broadcast_to([P, 4, HN]),
                                    )
                            bc_idx += 1
                            # bf16 path: rows 0..7 feed PE directly as bf16
                            ob16 = obf_pool.tile([P, 8, HN], bf16, tag="ob16")
                            nc.vector.tensor_mul(
                                ob16[:],
                                bc[:, :8, :],
                                h2tb_sb[:, None, n0 : n0 + HN].broadcast_to(
                                    [P, 8, HN]
                                ),
                            )
                            # fp8 path: rows 8..13 via bf16+ACT, 14..15 direct
                            o8 = o8_pool.tile([P, 8, HN], f8, tag="o8")
                            obf = obf_pool.tile([P, QA, HN], bf16, tag="obf")
                            nc.vector.tensor_mul(
                                obf[:],
                                bc[:, 8 : 8 + QA, :],
                                h2tb_sb[:, None, n0 : n0 + HN].broadcast_to(
                                    [P, QA, HN]
                                ),
                            )
                            nc.scalar.copy(o8[:, :QA, :], obf[:])
                            nc.vector.tensor_mul(
                                o8[:, QA:, :],
                                bc[:, 8 + QA :, :],
                                h2tb_sb[:, None, n0 : n0 + HN].broadcast_to(
                                    [P, 8 - QA, HN]
                                ),
                            )
                            for cl in range(8):
                                c = cb * CCH + cl
                                for m in range(3):
                                    nc.tensor.matmul(
                                        acc[m][:, hh * HN : (hh + 1) * HN],
                                        a16t[:, cl, m * P : (m + 1) * P],
                                        ob16[:, cl, :],
                                        start=(c == 0),
                                        stop=False,
                                    )
                            for pp in range(4):
                                for m in range(3):
                                    nc.tensor.matmul(
                                        acc[m][:, hh * HN : (hh + 1) * HN],
                                        a8t[:, pp, :, m * P : (m + 1) * P],
                                        o8[:, 2 * pp : 2 * pp + 2, :],
                                        start=False,
                                        stop=False,
                                        perf_mode=DR,
                                    )
                        if g == 0 and cb == 0:
                            _emit_setup()
                    # Linear terms, mean correction, bias, store (transposed).
                    n0 = g * GN
                    for m in range(3):
                        for hh in range(NH):
                            nn = n0 + hh * HN
                            sl = acc[m][:, hh * HN : (hh + 1) * HN]
                            nc.tensor.matmul(
                                sl,
                                u1t_sb[:, m, :],
                                h1tb_sb[:, nn : nn + HN],
                                start=False,
                                stop=False,
                            )
                            nc.tensor.matmul(
                                sl,
                                u2t_sb[:, m, :],
                                h2tb_sb[:, nn : nn + HN],
                                start=False,
                                stop=False,
                            )
                            nc.tensor.matmul(
                                sl,
                                half_sb[:],
                                gt_sb[:, nn : nn + HN].bitcast(f32r),
                                start=False,
                                stop=True,
                            )
                        osb = osb_pool.tile([P, GN], f32, tag="osb")
                        nc.scalar.activation(
                            osb[:], acc[m][:], AF.Identity, bias=bias_sb[:, m : m + 1]
                        )
                        for q in range(2):
                            nc.sync.dma_start(
                                out=outT.ap()[
                                    m * P : (m + 1) * P,
                                    n0 + q * HN : n0 + (q + 1) * HN,
                                ],
                                in_=osb[:, q * HN : (q + 1) * HN],
                            )

    nc.compile()
    return nc


def _get_nc():
    if "nc" not in _CACHE:
        _CACHE["nc"] = _build_nc()
    return _CACHE["nc"]


def _prep_full(inputs):
    import ml_dtypes

    f8 = ml_dtypes.float8_e4m3
    bf = ml_dtypes.bfloat16
    nhf = np.ascontiguousarray(np.asarray(inputs["neighbour_h"], dtype=np.float32))
    A = np.asarray(inputs["A"], dtype=np.float32)
    U1 = np.asarray(inputs["U1_w"], dtype=np.float32)
    U2 = np.asarray(inputs["U2_w"], dtype=np.float32)
    U2b = np.asarray(inputs["U2_b"], dtype=np.float32)

    Ac = (A - 0.5).astype(np.float32)
    # c-local rows 0..7 of each 16-chunk: bf16; rows 8..15: fp8 pairs
    Ac4 = Ac.reshape(NCB, CCH, H, KO)
    a16 = np.ascontiguousarray(
        Ac4[:, :8].transpose(2, 0, 1, 3)
    ).astype(bf)  # [j, cb, 8, KO]
    a8p = np.ascontiguousarray(
        Ac4[:, 8:].reshape(NCB, 4, 2, H, KO).transpose(3, 0, 1, 2, 4)
    ).astype(f8)  # [j, cb, 4, 2, KO]

    h1 = nhf[:, 0, :]
    h2 = nhf[:, 1, :]
    h1T = np.ascontiguousarray(h1.T).astype(bf)
    h2T = np.ascontiguousarray(h2.T).astype(bf)
    # h1bc[core][cb, g, h, c, n]
    h1bc = np.ascontiguousarray(
        h1T.reshape(NCB, CCH, N_CORES, GROUPS, NH, HN).transpose(2, 0, 3, 4, 1, 5)
    )

    u1t = np.ascontiguousarray(U1.reshape(3, P, H).transpose(2, 0, 1)).astype(bf)
    u2t = np.ascontiguousarray(U2.reshape(3, P, H).transpose(2, 0, 1)).astype(bf)
    biasw = np.ascontiguousarray(U2b.reshape(3, P).T)

    return nhf, a16, a8p, h1bc, h1T, h2T, u1t, u2t, biasw


def make_in_maps(inputs):
    nhf, a16, a8p, h1bc, h1T, h2T, u1t, u2t, biasw = _prep_full(inputs)
    return [
        {
            "a16": a16,
            "a8p": a8p,
            "h1bc": h1bc[i],
            "h2tb": np.ascontiguousarray(h2T[:, i * NS : (i + 1) * NS]),
            "h1tb": np.ascontiguousarray(h1T[:, i * NS : (i + 1) * NS]),
            "u1t": u1t,
            "u2t": u2t,
            "biasw": biasw,
            "neighbour_h": nhf[i * NS : (i + 1) * NS],
        }
        for i in range(N_CORES)
    ]


def kernel(**inputs: np.ndarray) -> np.ndarray:
    in_maps = make_in_maps(inputs)
    nc = _get_nc()
    from concourse import bass2jax

    results = bass2jax.run_bass_via_pjrt(nc, in_maps, n_cores=N_CORES)
    return np.concatenate(
        [np.asarray(results[i]["outT"], dtype=np.float32).T for i in range(N_CORES)],
        axis=0,
    )


if __name__ == "__main__":
    rng = np.random.default_rng(0)
    ins = {
        "neighbour_h": rng.standard_normal((N_FULL, 2, H), dtype=np.float32),
        "A": rng.random((H, H, KO), dtype=np.float32),
        "U1_w": rng.standard_normal((KO, H), dtype=np.float32),
        "U2_w": rng.standard_normal((KO, H), dtype=np.float32),
        "U2_b": rng.standard_normal((KO,), dtype=np.float32),
    }
    out = kernel(**ins)
    h1 = ins["neighbour_h"][:, 0, :].astype(np.float64)
    h2 = ins["neighbour_h"][:, 1, :].astype(np.float64)
    A = ins["A"].astype(np.float64)
    outer = np.einsum("ni,nj->nij", h1, h2).reshape(N_FULL, H * H)
    exp = (
        outer @ A.reshape(H * H, KO)
        + h1 @ ins["U1_w"].T.astype(np.float64)
        + h2 @ ins["U2_w"].T.astype(np.float64)
        + ins["U2_b"].astype(np.float64)
    )
    err = np.linalg.norm(out - exp) / np.linalg.norm(exp)
    print("kernel output", out.shape, out.dtype, "rel fro err:", err)


# revision 28
# speedup vs baseline: 1.3379x; 1.0112x over previous
"""Trainium2 Bass kernel for nn_BinaryFullTensorCell (gnn_message_passing).

Computes, for each node n:
    out[n,k] = sum_{i,j} h1[n,i]*h2[n,j]*A[i,j,k] + h1@U1_w.T + h2@U2_w.T + U2_b

Sharding: data-parallel over the node axis across 8 NeuronCores (2048
nodes/core); A / U1_w / U2_w / U2_b replicated.

Algorithm (v4, fp8e4 DoubleRow, single-e4m3-rounding error budget):
  - Host prep (layout/precision transforms of inputs only): A is
    mean-centered (A' = A - 0.5) and split into hi+lo fp8e4 terms in
    DoubleRow pair layout [j, pair, 2, k]; h1^T in bf16 broadcast-chunk
    layout; h2^T/h1^T/U1^T/U2^T in bf16.
  - outer_T[j, (c, n)] = h1T[c,n]*h2T[j,n] is built directly in
    transposed layout: h1T chunks are replicated across all 128
    partitions either by a stride-0 DMA broadcast or by GPSIMD
    PartitionBroadcast (split tunable via K_GPB), then multiplied by
    the partition-resident bf16 h2T on DVE.  A slice of c-rows is
    multiplied to bf16 at DVE 2x rate and quantized to fp8 on ACT; the
    rest is multiplied straight to fp8 on DVE (1x).  This is the only
    e4m3 rounding in the bilinear chain.
  - PE accumulates out_T[k, n] with fp8e4 DoubleRow matmuls against
    Ahi and Alo (A quantization error second order).
  - Mean restoration: bil += 0.5*s1[n]*s2[n] via g[n,j]=h2[n,j]*s1[n]
    (DVE), PE transpose, f32r matmul against a constant 0.5 matrix.
  - Linear terms as bf16 matmuls into the same PSUM group; bias added
    on ACT eviction.  Output stored transposed [3H, NS]; host
    un-transposes.
"""

import os

import numpy as np

N_FULL = 16384
N_CORES = 8
NS = N_FULL // N_CORES  # 2048 nodes per core
H = 128
KO = 3 * H  # 384
P = 128
GROUPS = 2  # groups of 1024 nodes per core
GN = NS // GROUPS  # 1024
NH = 2  # n-halves per group for bc/mult tiling
HN = GN // NH  # 512
NT = NS // P  # 16 node-tiles per core
CCH = 16  # contraction rows (c values) per broadcast/mult chunk
NCB = H // CCH  # 8 chunks
CPAIR = CCH // 2  # DoubleRow pairs per chunk

# Of each 16-c-row chunk, QA rows go DVE-mult(bf16,2x) -> ACT-quantize(fp8);
# the rest go DVE-mult straight to fp8 (1x).
# Per 16-c chunk: rows 0..7 bf16 path; rows 8..13 fp8 via ACT quantize;
# rows 14..15 fp8 via DVE direct multiply.
QA = int(os.environ.get("K_QA", "6"))
# Of every 16 bc tiles, K_GPB are broadcast by GPSIMD instead of DMA.
K_GPB = int(os.environ.get("K_GPB", "0"))
# PE warm-up matmuls at kernel start (DVFS ramp), overlapping initial DMAs.
WARMUP = int(os.environ.get("K_WARMUP", "0"))

_CACHE = {}


def _build_nc():
    import concourse.bacc as bacc
    import concourse.mybir as mybir
    import concourse.tile as tile
    from concourse.masks import make_identity

    f32 = mybir.dt.float32
    f32r = mybir.dt.float32r
    bf16 = mybir.dt.bfloat16
    f8 = mybir.dt.float8e4
    DR = mybir.MatmulPerfMode.DoubleRow
    AF = mybir.ActivationFunctionType
    nc = bacc.Bacc("TRN2", target_bir_lowering=False, debug=False)

    a16 = nc.dram_tensor("a16", [P, NCB, 8, KO], bf16, kind="ExternalInput")
    a8p = nc.dram_tensor("a8p", [P, NCB, 4, 2, KO], f8, kind="ExternalInput")
    # h1bc[cb, g, h, c, n] = h1T[cb*16+c, g*1024 + h*512 + n] (bf16)
    h1bc = nc.dram_tensor(
        "h1bc", [NCB, GROUPS, NH, CCH, HN], bf16, kind="ExternalInput"
    )
    h2tb = nc.dram_tensor("h2tb", [H, NS], bf16, kind="ExternalInput")
    h1tb = nc.dram_tensor("h1tb", [H, NS], bf16, kind="ExternalInput")
    u1t = nc.dram_tensor("u1t", [H, 3, P], bf16, kind="ExternalInput")
    u2t = nc.dram_tensor("u2t", [H, 3, P], bf16, kind="ExternalInput")
    biasw = nc.dram_tensor("biasw", [P, 3], f32, kind="ExternalInput")
    nh = nc.dram_tensor("neighbour_h", [NS, 2, H], f32, kind="ExternalInput")
    outT = nc.dram_tensor("outT", [KO, NS], f32, kind="ExternalOutput")

    with tile.TileContext(nc) as tc:
        with tc.tile_pool(name="consts", bufs=1) as consts:
            identity_r = consts.tile([P, P], f32r)
            half_sb = consts.tile([P, P], f32r)
            with tc.tile_pool(name="tmpconst", bufs=1) as tmpc:
                identity = tmpc.tile([P, P], f32)
                make_identity(nc, identity)
                nc.vector.tensor_copy(identity_r[:], identity[:])
                half_f32 = tmpc.tile([P, P], f32)
                nc.vector.memset(half_f32[:], 0.5)
                nc.vector.tensor_copy(half_sb[:], half_f32[:])

            h2tb_sb = consts.tile([P, NS], bf16)
            for q in range(2):
                nc.sync.dma_start(
                    out=h2tb_sb[:, q * 1024 : (q + 1) * 1024],
                    in_=h2tb.ap()[:, q * 1024 : (q + 1) * 1024],
                )
            h1tb_sb = consts.tile([P, NS], bf16)
            for q in range(2):
                nc.sync.dma_start(
                    out=h1tb_sb[:, q * 1024 : (q + 1) * 1024],
                    in_=h1tb.ap()[:, q * 1024 : (q + 1) * 1024],
                )
            u1t_sb = consts.tile([P, 3, P], bf16)
            nc.sync.dma_start(out=u1t_sb[:], in_=u1t.ap())
            u2t_sb = consts.tile([P, 3, P], bf16)
            nc.sync.dma_start(out=u2t_sb[:], in_=u2t.ap())
            bias_sb = consts.tile([P, 3], f32)
            nc.sync.dma_start(out=bias_sb[:], in_=biasw.ap())
            h_sb = consts.tile([P, NT, 2, H], f32)
            _nh_r = nh.ap().rearrange("(t p) a b -> p t a b", p=P)
            for q in range(4):
                nc.sync.dma_start(
                    out=h_sb[:, q * 4 : (q + 1) * 4], in_=_nh_r[:, q * 4 : (q + 1) * 4]
                )

            s1_sb = consts.tile([P, NT, 1], f32)
            g_sb = consts.tile([P, NT, H], f32r)
            gt_sb = consts.tile([P, NS], f32)

            with (
                tc.tile_pool(name="alo_pool", bufs=3) as alo_pool,
                tc.tile_pool(name="bc_pool", bufs=3) as bc_pool,
                tc.tile_pool(name="stage_pool", bufs=2) as stage_pool,
                tc.tile_pool(name="obf_pool", bufs=3) as obf_pool,
                tc.tile_pool(name="o8_pool", bufs=4) as o8_pool,
                tc.tile_pool(name="acc_ps", bufs=3, space="PSUM") as acc_ps,
                tc.tile_pool(name="tr_ps", bufs=2, space="PSUM") as tr_ps,
                tc.tile_pool(name="osb_pool", bufs=2) as osb_pool,
            ):
                def _emit_setup():
                    nc.vector.tensor_reduce(
                        s1_sb[:],
                        h_sb[:, :, 0, :],
                        mybir.AxisListType.X,
                        mybir.AluOpType.add,
                    )
                    nc.vector.tensor_mul(
                        g_sb[:],
                        h_sb[:, :, 1, :],
                        s1_sb[:].broadcast_to([P, NT, H]),
                    )
                    for tq in range(NT // 4):
                        ps = tr_ps.tile([P, 512], f32r, tag="tr", name=f"gt{tq}")
                        for tt in range(4):
                            t = tq * 4 + tt
                            nc.tensor.transpose(
                                ps[:, tt * P : (tt + 1) * P],
                                g_sb[:, t, :],
                                identity_r[:],
                            )
                        nc.vector.tensor_copy(
                            gt_sb[:, tq * 512 : (tq + 1) * 512].bitcast(f32r), ps[:]
                        )

                # PE warm-up: back-to-back DoubleRow matmuls on constant
                # data to push DVFS to the high p-state while the initial
                # DMAs land.  Results are discarded (overwritten by start=True
                # of the real accumulation later via pool reuse).
                if WARMUP:
                    wsrc = consts.tile([P, 2, 256], f8)
                    nc.vector.memset(wsrc[:], 0.25)
                    wps = tr_ps.tile([P, 512], f32, tag="tr", name="warm")
                    for w in range(WARMUP):
                        nc.tensor.matmul(
                            wps[:, :256],
                            wsrc[:, :, :P],
                            wsrc[:],
                            start=True,
                            stop=True,
                            perf_mode=DR,
                        )

                bc_idx = 0
                for g in range(GROUPS):
                    acc = [
                        acc_ps.tile([P, GN], f32, tag="acc", name=f"acc{g}_{m}")
                        for m in range(3)
                    ]
                    for cb in range(NCB):
                        a16t = alo_pool.tile([P, 8, KO], bf16, tag="a16")
                        for q in range(4):
                            nc.sync.dma_start(
                                out=a16t[:, q * 2 : (q + 1) * 2],
                                in_=a16.ap()[:, cb, q * 2 : (q + 1) * 2],
                            )
                        a8t = alo_pool.tile([P, 4, 2, KO], f8, tag="a8")
                        for q in range(2):
                            nc.sync.dma_start(
                                out=a8t[:, q * 2 : (q + 1) * 2],
                                in_=a8p.ap()[:, cb, q * 2 : (q + 1) * 2],
                            )
                        for hh in range(NH):
                            n0 = g * GN + hh * HN
                            bc = bc_pool.tile([P, CCH, HN], bf16, tag="bc")
                            if bc_idx % 16 < K_GPB:
                                HL = CCH * HN // 2
                                for sh in range(2):
                                    stage = stage_pool.tile([1, HL], bf16, tag="st")
                                    nc.sync.dma_start(
                                        out=stage[:],
                                        in_=h1bc.ap()[cb, g, hh]
                                        .rearrange("c n -> (c n)")
                                        .unsqueeze(0)[:, sh * HL : (sh + 1) * HL],
                                    )
                                    nc.gpsimd.partition_broadcast(
                                        bc[:].rearrange("p c n -> p (c n)")[
                                            :, sh * HL : (sh + 1) * HL
                                        ],
                                        stage[0:1, :],
                                    )
                            else:
                                for q in range(4):
                                    nc.sync.dma_start(
                                        out=bc[:, q * 4 : (q + 1) * 4, :],
                                        in_=h1bc.ap()[cb, g, hh][
                                            q * 4 : (q + 1) * 4
                                        ]
                                        .unsqueeze(0)
                                        .broadcast_to([P, 4, HN]),
                                    )
                            bc_idx += 1
                            # bf16 path: rows 0..7 feed PE directly as bf16
                            ob16 = obf_pool.tile([P, 8, HN], bf16, tag="ob16")
                            nc.vector.tensor_mul(
                                ob16[:],
                                bc[:, :8, :],
                                h2tb_sb[:, None, n0 : n0 + HN].broadcast_to(
                                    [P, 8, HN]
                                ),
                            )
                            # fp8 path: rows 8..13 via bf16+ACT, 14..15 direct
                            o8 = o8_pool.tile([P, 8, HN], f8, tag="o8")
                            obf = obf_pool.tile([P, QA, HN], bf16, tag="obf")
                            nc.vector.tensor_mul(
                                obf[:],
                                bc[:, 8 : 8 + QA, :],
                                h2tb_sb[:, None, n0 : n0 + HN].broadcast_to(
                                    [P, QA, HN]
                                ),
                            )
                            nc.scalar.copy(o8[:, :QA, :], obf[:])
                            nc.vector.tensor_mul(
                                o8[:, QA:, :],
                                bc[:, 8 + QA :, :],
                                h2tb_sb[:, None, n0 : n0 + HN].broadcast_to(
                                    [P, 8 - QA, HN]
                                ),
                            )
                            for cl in range(8):
                                c = cb * CCH + cl
                                for m in range(3):
                                    nc.tensor.matmul(
                                        acc[m][:, hh * HN : (hh + 1) * HN],
                                        a16t[:, cl, m * P : (m + 1) * P],
                                        ob16[:, cl, :],
                                        start=(c == 0),
                                        stop=False,
                                    )
                            for pp in range(4):
                                for m in range(3):
                                    nc.tensor.matmul(
                                        acc[m][:, hh * HN : (hh + 1) * HN],
                                        a8t[:, pp, :, m * P : (m + 1) * P],
                                        o8[:, 2 * pp : 2 * pp + 2, :],
                                        start=False,
                                        stop=False,
                                        perf_mode=DR,
                                    )
                        if g == 0 and cb == 0:
                            _emit_setup()
                    # Linear terms, mean correction, bias, store (transposed).
                    n0 = g * GN
                    for m in range(3):
                        for hh in range(NH):
                            nn = n0 + hh * HN
                            sl = acc[m][:, hh * HN : (hh + 1) * HN]
                            nc.tensor.matmul(
                                sl,
                                u1t_sb[:, m, :],
                                h1tb_sb[:, nn : nn + HN],
                                start=False,
                                stop=False,
                            )
                            nc.tensor.matmul(
                                sl,
                                u2t_sb[:, m, :],
                                h2tb_sb[:, nn : nn + HN],
                                start=False,
                                stop=False,
                            )
                            nc.tensor.matmul(
                                sl,
                                half_sb[:],
                                gt_sb[:, nn : nn + HN].bitcast(f32r),
                                start=False,
                                stop=True,
                            )
                        osb = osb_pool.tile([P, GN], f32, tag="osb")
                        nc.scalar.activation(
                            osb[:], acc[m][:], AF.Identity, bias=bias_sb[:, m : m + 1]
                        )
                        for q in range(2):
                            nc.sync.dma_start(
                                out=outT.ap()[
                                    m * P : (m + 1) * P,
                                    n0 + q * HN : n0 + (q + 1) * HN,
                                ],
                                in_=osb[:, q * HN : (q + 1) * HN],
                            )

    nc.compile()
    return nc


def _get_nc():
    if "nc" not in _CACHE:
        _CACHE["nc"] = _build_nc()
    return _CACHE["nc"]


def _prep_full(inputs):
    import ml_dtypes

    f8 = ml_dtypes.float8_e4m3
    bf = ml_dtypes.bfloat16
    nhf = np.ascontiguousarray(np.asarray(inputs["neighbour_h"], dtype=np.float32))
    A = np.asarray(inputs["A"], dtype=np.float32)
    U1 = np.asarray(inputs["U1_w"], dtype=np.float32)
    U2 = np.asarray(inputs["U2_w"], dtype=np.float32)
    U2b = np.asarray(inputs["U2_b"], dtype=np.float32)

    Ac = (A - 0.5).astype(np.float32)
    # c-local rows 0..7 of each 16-chunk: bf16; rows 8..15: fp8 pairs
    Ac4 = Ac.reshape(NCB, CCH, H, KO)
    a16 = np.ascontiguousarray(
        Ac4[:, :8].transpose(2, 0, 1, 3)
    ).astype(bf)  # [j, cb, 8, KO]
    a8p = np.ascontiguousarray(
        Ac4[:, 8:].reshape(NCB, 4, 2, H, KO).transpose(3, 0, 1, 2, 4)
    ).astype(f8)  # [j, cb, 4, 2, KO]

    h1 = nhf[:, 0, :]
    h2 = nhf[:, 1, :]
    h1T = np.ascontiguousarray(h1.T).astype(bf)
    h2T = np.ascontiguousarray(h2.T).astype(bf)
    # h1bc[core][cb, g, h, c, n]
    h1bc = np.ascontiguousarray(
        h1T.reshape(NCB, CCH, N_CORES, GROUPS, NH, HN).transpose(2, 0, 3, 4, 1, 5)
    )

    u1t = np.ascontiguousarray(U1.reshape(3, P, H).transpose(2, 0, 1)).astype(bf)
    u2t = np.ascontiguousarray(U2.reshape(3, P, H).transpose(2, 0, 1)).astype(bf)
    biasw = np.ascontiguousarray(U2b.reshape(3, P).T)

    return nhf, a16, a8p, h1bc, h1T, h2T, u1t, u2t, biasw


def make_in_maps(inputs):
    nhf, a16, a8p, h1bc, h1T, h2T, u1t, u2t, biasw = _prep_full(inputs)
    return [
        {
            "a16": a16,
            "a8p": a8p,
            "h1bc": h1bc[i],
            "h2tb": np.ascontiguousarray(h2T[:, i * NS : (i + 1) * NS]),
            "h1tb": np.ascontiguousarray(h1T[:, i * NS : (i + 1) * NS]),
            "u1t": u1t,
            "u2t": u2t,
            "biasw": biasw,
            "neighbour_h": nhf[i * NS : (i + 1) * NS],
        }
        for i in range(N_CORES)
    ]


def kernel(**inputs: np.ndarray) -> np.ndarray:
    in_maps = make_in_maps(inputs)
    nc = _get_nc()
    from concourse import bass2jax

    results = bass2jax.run_bass_via_pjrt(nc, in_maps, n_cores=N_CORES)
    return np.concatenate(
        [np.asarray(results[i]["outT"], dtype=np.float32).T for i in range(N_CORES)],
        axis=0,
    )


if __name__ == "__main__":
    rng = np.random.default_rng(0)
    ins = {
        "neighbour_h": rng.standard_normal((N_FULL, 2, H), dtype=np.float32),
        "A": rng.random((H, H, KO), dtype=np.float32),
        "U1_w": rng.standard_normal((KO, H), dtype=np.float32),
        "U2_w": rng.standard_normal((KO, H), dtype=np.float32),
        "U2_b": rng.standard_normal((KO,), dtype=np.float32),
    }
    out = kernel(**ins)
    h1 = ins["neighbour_h"][:, 0, :].astype(np.float64)
    h2 = ins["neighbour_h"][:, 1, :].astype(np.float64)
    A = ins["A"].astype(np.float64)
    outer = np.einsum("ni,nj->nij", h1, h2).reshape(N_FULL, H * H)
    exp = (
        outer @ A.reshape(H * H, KO)
        + h1 @ ins["U1_w"].T.astype(np.float64)
        + h2 @ ins["U2_w"].T.astype(np.float64)
        + ins["U2_b"].astype(np.float64)
    )
    err = np.linalg.norm(out - exp) / np.linalg.norm(exp)
    print("kernel output", out.shape, out.dtype, "rel fro err:", err)
